# revision 1
# baseline (speedup 1.0000x reference)
"""Trainium2 Bass kernel for nn_BiLSTM_58351425683848.

Self-contained: accepts the FULL inputs of reference.setup_inputs(), returns
the FULL [256, 1024] output. Internally row-shards the sequence dim across 8
NeuronCores (the only cross-core data each step is the 16KB of BatchNorm
statistics, exchanged with two 8KB AllGathers); all GEMMs run in fp16 hi/lo
split arithmetic (fp32-equivalent accuracy at ~4x fp32 PE throughput).

Only the live part of the reference is computed: the LSTM cell updates, W4/b4
and the per-step outputs are dead code — the result is out[-1] =
0.5*(hf2+hb2) at t=255 of the interaction/BatchNorm recurrence.
"""
import sys
sys.path.insert(0, '/opt/trn_rl_repo')
import numpy as np

S = 256
H = 1024
EPS = 1e-5
NK = 8
SL = 32


def pack_actT(a):
    sl = a.shape[0]
    out = np.empty((128, NK * sl), a.dtype)
    for k in range(NK):
        out[:, k * sl:(k + 1) * sl] = a[:, k * 128:(k + 1) * 128].T
    return np.ascontiguousarray(out)


def unpack_actT(p, sl=SL):
    a = np.empty((sl, H), p.dtype)
    for k in range(NK):
        a[:, k * 128:(k + 1) * 128] = p[:, k * sl:(k + 1) * sl].T
    return a


def pack_w_moving(w):
    out = np.empty((128, NK * H), w.dtype)
    for k in range(NK):
        out[:, k * H:(k + 1) * H] = w[:, k * 128:(k + 1) * 128].T
    return np.ascontiguousarray(out)


def pack_vec(v):
    return np.ascontiguousarray(v.reshape(NK, 128).T)


def split16(x):
    hi = x.astype(np.float16)
    lo = (x - hi.astype(np.float32)).astype(np.float16)
    return hi, lo


def build_kernel(nsteps, n_cores=8, debug_taps=()):
    import sys
    sys.path.insert(0, '/opt/trn_rl_repo')
    import concourse.bacc as bacc
    import concourse.tile as tile
    import concourse.mybir as mybir

    f32 = mybir.dt.float32
    f16 = mybir.dt.float16
    AFT = mybir.ActivationFunctionType
    ALU = mybir.AluOpType

    nc = bacc.Bacc("TRN2", target_bir_lowering=False, debug=False,
                   num_devices=n_cores)

    xt = nc.dram_tensor("xt", [128, NK * SL], f32, kind="ExternalInput")
    w_in = {}
    for nm in ("w1h", "w1l", "w2h", "w2l", "w3h", "w3l"):
        w_in[nm] = nc.dram_tensor(nm, [128, NK * H], f16, kind="ExternalInput")
    # bias rows: (b_hi; b_lo) [2, 1024] per gemm -> packed [2, 3*1024] fp16
    brow_in = nc.dram_tensor("brow", [2, 3 * H], f16, kind="ExternalInput")
    vecs_in = nc.dram_tensor("vecs", [128, NK * 6], f32, kind="ExternalInput")
    # vecs: gf bf gb bb (4 used of 6)
    outp = nc.dram_tensor("out", [128, NK * SL], f32, kind="ExternalOutput")
    taps = {}
    for nm in debug_taps:
        taps[nm] = nc.dram_tensor(f"tap_{nm}", [128, NK * SL], f32,
                                  kind="ExternalOutput")

    E_np = np.tile(np.eye(SL, dtype=np.float32), (4, 1))
    e_dram = nc.inline_tensor(E_np, name="emat")
    ones2_np = np.ones((2, SL), dtype=np.float16)
    ones2_dram = nc.inline_tensor(ones2_np, name="ones2")

    with tile.TileContext(nc) as tc:
        with tc.tile_pool(name="wpool", bufs=1) as wpool, \
             tc.tile_pool(name="spool", bufs=3) as spool, \
             tc.tile_pool(name="dpool", bufs=4, space="DRAM") as dpool, \
             tc.tile_pool(name="ppool", bufs=2, space="PSUM") as ppool, \
             tc.tile_pool(name="pxpool", bufs=2, space="PSUM") as pxpool, \
             tc.tile_pool(name="warmp", bufs=1, space="PSUM") as warmp:

            w_sb = {}
            for nm in w_in:
                w_sb[nm] = wpool.tile([128, NK * H], f16, tag=nm, name=nm)
                for k in range(NK):
                    nc.sync.dma_start(w_sb[nm][:, k * H:(k + 1) * H],
                                      w_in[nm][:, k * H:(k + 1) * H])
            brow = wpool.tile([2, 3 * H], f16, tag="brow")
            nc.sync.dma_start(brow[:], brow_in[:])
            vecs = wpool.tile([128, NK * 6], f32, tag="vecs")
            nc.sync.dma_start(vecs[:], vecs_in[:])
            gfp = vecs[:, 0 * NK:1 * NK]
            bfp = vecs[:, 1 * NK:2 * NK]
            gbp = vecs[:, 2 * NK:3 * NK]
            bbp = vecs[:, 3 * NK:4 * NK]
            e_sb = wpool.tile([128, SL], f32, tag="emat")
            nc.sync.dma_start(e_sb[:], e_dram[:])
            ones2 = wpool.tile([2, SL], f16, tag="ones2")
            nc.sync.dma_start(ones2[:], ones2_dram[:])
            xt_sb = wpool.tile([128, NK * SL], f32, tag="xt")
            nc.sync.dma_start(xt_sb[:], xt[:])
            epsc = wpool.tile([128, 1], f32, tag="epsc")
            nc.vector.memset(epsc[:], EPS)

            hfT = wpool.tile([128, NK * SL], f32, tag="hfT")
            hbT = wpool.tile([128, NK * SL], f32, tag="hbT")
            nc.vector.memset(hfT[:], 0.0)
            nc.vector.memset(hbT[:], 0.0)

            PW = NK * SL

            def split_a(a, tagbase):
                ah = spool.tile([128, PW], f16, tag=tagbase + "h",
                                name=tagbase + "h")
                al = spool.tile([128, PW], f16, tag=tagbase + "l",
                                name=tagbase + "l")
                nc.vector.tensor_copy(ah[:], a[:])
                nc.vector.tensor_sub(al[:], a[:], ah[:])
                return ah, al

            def add_split(x, y, tagbase):
                """(ah, al) = fp16 hi/lo of (x + y), no fp32 materialization."""
                ah = spool.tile([128, PW], f16, tag=tagbase + "h",
                                name=tagbase + "h")
                al = spool.tile([128, PW], f16, tag=tagbase + "l",
                                name=tagbase + "l")
                tr = spool.tile([128, PW], f32, tag="addres", name="addres")
                nc.vector.tensor_add(ah[:], x[:], y[:])
                nc.vector.tensor_sub(tr[:], x[:], ah[:])
                nc.vector.tensor_add(al[:], tr[:], y[:])
                return ah, al

            def gemm(ah, al, wh, wl, bslice, px, copy_engine):
                """PSUM <- Ah@Wh + Ah@Wl + Al@Wh + bias; E-matmul into px."""
                P = ppool.tile([128, H], f32, tag="P", name="P")
                units = [(k, p) for k in range(NK) for p in range(3)]
                per_group = [[] for _ in range(4)]
                for ui, u in enumerate(units):
                    per_group[ui % 4].append(u)
                per_group[0].insert(0, "bias")
                for g in range(4):
                    lst = per_group[g]
                    for idx, u in enumerate(lst):
                        first = (idx == 0)
                        last_u = (idx == len(lst) - 1)
                        if u == "bias":
                            lhs_ap = ones2[:]
                            w_ap = lambda nh: bslice[:, 512 * nh:512 * (nh + 1)]
                        else:
                            k, p = u
                            lhs = (ah if p in (0, 1) else al)
                            w = (w_sb[wh] if p in (0, 2) else w_sb[wl])
                            lhs_ap = lhs[:, k * SL:(k + 1) * SL]
                            w_ap = (lambda nh, w=w, k=k:
                                    w[:, k * H + 512 * nh:k * H + 512 * (nh + 1)])
                        for nh in range(2):
                            nc.tensor.matmul(
                                P[32 * g:32 * (g + 1), 512 * nh:512 * (nh + 1)],
                                lhs_ap,
                                w_ap(nh) if callable(w_ap) else w_ap,
                                start=first, stop=last_u,
                                tile_position=(0, 32 * g),
                            )
                Ssb = spool.tile([128, H], f32, tag="Ssb", name="Ssb")
                if copy_engine == "act":
                    nc.scalar.activation(Ssb[:], P[:], AFT.Copy)
                else:
                    nc.vector.tensor_copy(Ssb[:], P[:])
                for j in range(NK):
                    nc.tensor.matmul(
                        px[:, j * SL:(j + 1) * SL],
                        Ssb[:, j * 128:(j + 1) * 128],
                        e_sb[:],
                        start=True, stop=True,
                    )
                return px

            def stats_of(hx2, tag):
                # returns [128,16] tile: cols 0:8 sums, 8:16 sumsq
                st = spool.tile([128, 16], f32, tag="st" + tag, name="st" + tag)
                nc.vector.tensor_reduce(st[:, 0:8],
                                        hx2[:].rearrange("p (j s) -> p j s", j=NK),
                                        axis=mybir.AxisListType.X, op=ALU.add)
                sq_ = spool.tile([128, PW], f32, tag="sqscr", name="sq" + tag)
                nc.scalar.activation(sq_[:], hx2[:], AFT.Square)
                nc.vector.tensor_reduce(st[:, 8:16],
                                        sq_[:].rearrange("p (j s) -> p j s", j=NK),
                                        axis=mybir.AxisListType.X, op=ALU.add)
                return st

            def launch_ag(st, tag):
                inb = dpool.tile([128, 16], f32, tag="agi" + tag,
                                 name="agi" + tag)
                outb = dpool.tile([128 * n_cores, 16], f32, tag="ago" + tag,
                                  name="ago" + tag)
                nc.sync.dma_start(inb[:], st[:])
                nc.gpsimd.collective_compute(
                    "AllGather", ALU.bypass,
                    replica_groups=[list(range(n_cores))],
                    ins=[inb.opt()], outs=[outb.opt()],
                )
                return outb

            def bn_apply(outb, gamma, beta, hx2, hxT, tag):
                """Gather -> totals -> params -> hxT = a*hx2 + c."""
                gath = spool.tile([128, n_cores * 16], f32, tag="gath" + tag,
                                  name="gath" + tag)
                nc.sync.dma_start(
                    gath[:].rearrange("p (r c) -> p r c", r=n_cores),
                    outb[:].rearrange("(r p) c -> p r c", p=128))
                tot = spool.tile([128, 16], f32, tag="tot" + tag,
                                 name="tot" + tag)
                nc.vector.tensor_reduce(
                    tot[:], gath[:].rearrange("p (r c) -> p c r", r=n_cores),
                    axis=mybir.AxisListType.X, op=ALU.add)
                prm = spool.tile([128, 40], f32, tag="prm" + tag,
                                 name="prm" + tag)
                mean = prm[:, 0:8]
                var = prm[:, 8:16]
                a_ = prm[:, 16:24]
                c_ = prm[:, 24:32]
                msq = prm[:, 32:40]
                nc.vector.tensor_scalar_mul(mean, tot[:, 0:8], 1.0 / S)
                nc.vector.tensor_mul(msq, mean, mean)
                nc.vector.tensor_scalar(var, tot[:, 8:16], 1.0 / S, None,
                                        ALU.mult)
                nc.vector.tensor_sub(var, var, msq)
                sq = spool.tile([128, 8], f32, tag="sqv" + tag,
                                name="sqv" + tag)
                nc.scalar.activation(sq[:], var, AFT.Sqrt, bias=epsc[:, 0:1])
                r0 = spool.tile([128, 8], f32, tag="r0" + tag,
                                name="r0" + tag)
                nc.vector.reciprocal(r0[:], sq[:])
                nr = spool.tile([128, 24], f32, tag="nr" + tag,
                                name="nr" + tag)
                nc.vector.tensor_mul(nr[:, 0:8], r0[:], r0[:])
                nc.vector.tensor_scalar(nr[:, 8:16], var, EPS, -0.5,
                                        ALU.add, ALU.mult)
                nc.vector.tensor_mul(nr[:, 0:8], nr[:, 0:8], nr[:, 8:16])
                nc.vector.tensor_scalar_add(nr[:, 0:8], nr[:, 0:8], 1.5)
                nc.vector.tensor_mul(r0[:], r0[:], nr[:, 0:8])
                nc.vector.tensor_mul(a_, gamma, r0[:])
                nc.vector.tensor_mul(c_, a_, mean)
                nc.vector.tensor_sub(c_, beta, c_)
                for j in range(NK):
                    nc.vector.tensor_scalar(
                        hxT[:, j * SL:(j + 1) * SL],
                        hx2[:, j * SL:(j + 1) * SL],
                        a_[:, j:j + 1], c_[:, j:j + 1],
                        ALU.mult, ALU.add)

            # ---- pipelined main loop ----
            # carried across iterations: pending AG_b + hb2 of previous step
            pend_b = None  # (outb, hb2_tile)
            for t in range(nsteps):
                last = (t == nsteps - 1)
                # G1 (PE busy while pending AG_b is in flight)
                a1h, a1l = add_split(xt_sb, hfT, "a1s")
                px1 = pxpool.tile([128, PW], f32, tag="px", name="px1")
                gemm(a1h, a1l, "w1h", "w1l", brow[:, 0:H], px1, "act")
                x1 = spool.tile([128, PW], f32, tag="x1")
                nc.scalar.activation(x1[:], px1[:], AFT.Sigmoid)

                # finish previous step's backward BN (overlaps G1's MMs)
                if pend_b is not None:
                    outb_b, hb2_prev = pend_b
                    bn_apply(outb_b, gbp, bbp, hb2_prev, hbT, "b")
                    pend_b = None

                # G3 (forward)
                a3h, a3l = add_split(x1, hfT, "a3s")
                px3 = pxpool.tile([128, PW], f32, tag="px", name="px3")
                gemm(a3h, a3l, "w3h", "w3l", brow[:, 2 * H:3 * H], px3, "act")
                hf2 = spool.tile([128, PW], f32, tag="hf2")
                nc.scalar.activation(hf2[:], px3[:], AFT.Sigmoid)
                if not last:
                    st_f = stats_of(hf2, "f")
                    outb_f = launch_ag(st_f, "f")

                # G2 (backward) — PE busy while AG_f in flight
                a2h, a2l = add_split(hbT, x1, "a2s")
                px2 = pxpool.tile([128, PW], f32, tag="px", name="px2")
                gemm(a2h, a2l, "w2h", "w2l", brow[:, H:2 * H], px2, "act")
                hb2 = spool.tile([128, PW], f32, tag="hb2")
                nc.scalar.activation(hb2[:], px2[:], AFT.Sigmoid)

                if last:
                    o = spool.tile([128, PW], f32, tag="o")
                    nc.vector.tensor_add(o[:], hf2[:], hb2[:])
                    nc.vector.tensor_scalar_mul(o[:], o[:], 0.5)
                    nc.sync.dma_start(outp[:], o[:])
                    for nm, ap in (("x1", x1), ("hf2", hf2), ("hb2", hb2)):
                        if nm in taps:
                            nc.sync.dma_start(taps[nm][:], ap[:])
                    continue

                st_b = stats_of(hb2, "b")
                outb_b = launch_ag(st_b, "b")
                pend_b = (outb_b, hb2)

                # keep-warm: dummy MMs anchored on a2h fill the AG_f wait so
                # the PE's HAM clock gate stays at 2.4GHz across the gap
                wp = warmp.tile([128, 512], f32, tag="wp", name="wp")
                for d in range(8):
                    nc.tensor.matmul(wp[0:32, :],
                                     a2h[:, (d % NK) * SL:((d % NK) + 1) * SL],
                                     w_sb["w1h"][:, 0:512],
                                     start=True, stop=True,
                                     skip_group_check=True)
                wscr = spool.tile([128, 8], f32, tag="wscr", name="wscr")
                nc.vector.tensor_copy(wscr[:32, :], wp[0:32, 0:8])

                # forward BN for next step's G1 (AG_f should have landed)
                bn_apply(outb_f, gfp, bfp, hf2, hfT, "f")

    nc.compile()
    return nc


def numpy_sim(inp, nsteps):
    sig = lambda x: 1.0 / (1.0 + np.exp(-x))

    def bn(x, g, b):
        m = x.mean(0)
        xc = x - m
        v = (xc * xc).mean(0)
        return xc / np.sqrt(v + EPS) * g + b

    X = inp["inputs"]
    hf = np.zeros((S, H), np.float32)
    hb = np.zeros((S, H), np.float32)
    for t in range(nsteps):
        x1 = sig((X + hf) @ inp["W1"].T + inp["b1"])
        hb2 = sig((hb + x1) @ inp["W2"].T + inp["b2"])
        hf2 = sig((x1 + hf) @ inp["W3"].T + inp["b3"])
        out = (hf2 + hb2) * 0.5
        hf = bn(hf2, inp["gamma_f"], inp["beta_f"])
        hb = bn(hb2, inp["gamma_b"], inp["beta_b"])
    return out, x1, hf2, hb2


def make_in_maps(inp, n_cores=8):
    m = {}
    for i, wn in enumerate(("W1", "W2", "W3")):
        wh, wl = split16(np.asarray(inp[wn], np.float32))
        m[f"w{i+1}h"] = pack_w_moving(wh)
        m[f"w{i+1}l"] = pack_w_moving(wl)
    brow = np.zeros((2, 3 * H), np.float16)
    for i, bn_ in enumerate(("b1", "b2", "b3")):
        bh, bl = split16(np.asarray(inp[bn_], np.float32))
        brow[0, i * H:(i + 1) * H] = bh
        brow[1, i * H:(i + 1) * H] = bl
    m["brow"] = brow
    vecs = np.zeros((128, NK * 6), np.float32)
    for i, nm in enumerate(("gamma_f", "beta_f", "gamma_b", "beta_b")):
        vecs[:, i * NK:(i + 1) * NK] = pack_vec(np.asarray(inp[nm], np.float32))
    m["vecs"] = vecs
    X = np.asarray(inp["inputs"], np.float32)
    maps = []
    for c in range(n_cores):
        mm = dict(m)
        mm["xt"] = pack_actT(X[c * SL:(c + 1) * SL, :])
        maps.append(mm)
    return maps


def assemble_out(results, n_cores=8):
    out = np.empty((S, H), np.float32)
    for c in range(n_cores):
        out[c * SL:(c + 1) * SL, :] = unpack_actT(results[c]["out"])
    return out


_NC_CACHE = {}


def kernel(**inputs):
    import numpy as np
    nsteps = S  # 256 scan steps
    key = nsteps
    if key not in _NC_CACHE:
        _NC_CACHE[key] = build_kernel(nsteps)
    nc = _NC_CACHE[key]
    inp = {k: np.asarray(v) for k, v in inputs.items()}
    maps = make_in_maps(inp)
    from concourse.bass_utils import run_bass_kernel_spmd
    res = run_bass_kernel_spmd(nc, maps, core_ids=list(range(8)))
    return assemble_out(res.results).astype(np.float32)

